# revision 29
# baseline (speedup 1.0000x reference)
"""DotProductPredictor kernel for trn2 (8 NeuronCores, SPMD).

Computes per-edge dot products score[e] = <h[src[e]], h[dst[e]]> over 600k
edges against a 100k x 128 fp32 node table; the final output
(score != global_min(score)) as float32 [600000, 1] matches the
reference's min-max normalize + (norm==0 ? 0 : 1) threshold.

Device strategy: edges sharded 8-way data-parallel; h replicated. Row
gathers use the GPSIMD dma_gather custom instruction (int16 indices), so h
is split into 4 banks of 25000 rows and each core's edges are grouped by
(src_bank, dst_bank) into 16 groups of a fixed 5120-edge capacity. Pads up
to the SPMD-uniform per-group count are duplicate in-group edges; slots
beyond it carry index -1, which the ucode trims (no descriptor / no DMA
read). Gathers are spread round-robin over 4 SWDGE queues (per-gather Q7
pair cost is ~6ns/idx + ~2.6us fixed, queue-independent; every instruction
also needs all 8 Q7 cores to idle-respond, so equal shares win). h is cast
to bf16 host-side (min-gap 2.67 dwarfs bf16 noise ~0.05; the threshold
compares our own f32 scores to their own min, so the argmin is preserved).
DVE multiply + per-edge segmented reduce produce f32 scores; the device
ships raw scores and the host does the global min + (score != min)
threshold (plus the slot unpermutation it already did).
"""

import os

import numpy as np

from concourse import bass, mybir, tile
from concourse import library_config
from concourse.bass_utils import run_bass_kernel_spmd

P = 128            # SBUF partitions
D = 128            # feature dim (one h row = 512B)
N_NODES = 100000
E_TOTAL = 600000
N_CORES = 8
EPC = E_TOTAL // N_CORES       # 75000 edges per core

N_BANKS = 4
BANK = N_NODES // N_BANKS      # 25000 rows per bank (< 32768 => int16 ok)
N_GROUPS = N_BANKS * N_BANKS   # 16 (src_bank, dst_bank) groups
GROUP_CAP = 5120               # fixed per-group slot allocation (mean 4687)
# Per-gather Q7 pair cost is ~6ns/idx + ~2.6us fixed (HW-measured). Bigger
# chunks would amortize the fixed cost, but the dma_gather ucode stack
# breaks above 1024 indices/instruction (device hangs at 2048 and 5120,
# with either packeting mode and bigger descriptor rings).
CHUNK = int(os.environ.get("KERNEL_CHUNK", "1024"))
CB = CHUNK // P                # score blocks per chunk
N_CHUNKS = N_GROUPS * GROUP_CAP // CHUNK
SLOTS = N_GROUPS * GROUP_CAP   # 81920 padded edge slots per core
SCORE_COLS = SLOTS // P        # 640
IDX_COLS = CHUNK // 16         # int16 columns per gather
N_GATHERS = 2 * N_CHUNKS
DVE_W = int(os.environ.get("KERNEL_DVE_W", "1024"))  # DVE op width (lock window)

_CACHE = {}


N_SWDGE_QUEUES = int(os.environ.get("KERNEL_SWDGE_QUEUES", "4"))
H_BF16 = os.environ.get("KERNEL_H_BF16", "1") == "1"
# Per-descriptor packets: the concatenated-stream mode would put 65 descs
# in one packet (over the documented 64-desc ceiling) and measured slightly
# slower; per-desc packets also interleave better across the 4 queues.
SINGLE_PACKET = os.environ.get("KERNEL_SINGLE_PACKET", "0") == "1"
DMA_SCRATCH = int(os.environ.get("KERNEL_DMA_SCRATCH", "16384"))
GBUFS = int(os.environ.get("KERNEL_GBUFS", "4"))
MBUFS = int(os.environ.get("KERNEL_MBUFS", "4"))
IDX_SPLITS = int(os.environ.get("KERNEL_IDX_SPLITS", "1"))
# bf16 product tile: 16-bit tensor_tensor runs at 2x on DVE, halving the
# window during which the DVE 2-input op holds the shared SBUF port pair
# that Q7 (SWDGE descriptor generation) needs. BUT a bf16-input
# tensor_reduce then runs in 16-bit 2x perf mode which ALSO grabs the
# shared pair (f32-input reduce does not), a net loss — so default off.
M_BF16 = os.environ.get("KERNEL_M_BF16", "0") == "1"
# Diagnostic: skip all DVE work to measure pure gather throughput.
SKIP_DVE = os.environ.get("KERNEL_SKIP_DVE", "0") == "1"

# Per-queue gather cost model (ns): the dma_gather ucode reads its int16
# index tile with channels=(queue+1)*32 active partitions starting at
# partition 0 (covering up to the issuing Q7 pair), so higher queue numbers
# pay a linearly larger idx-drain. Measured on HW (CHUNK=1024, no DVE):
# T(q) = B + KAPPA * 32*(q+1) * (CHUNK/16).
Q_BASE_NS = float(os.environ.get("KERNEL_QBASE", "2570"))
Q_KAPPA = float(os.environ.get("KERNEL_QKAPPA", "0.579"))


def queue_cost_ns(q, chunk):
    return Q_BASE_NS + Q_KAPPA * 32 * (q + 1) * (chunk / 16)


def build_queue_schedule(n_gathers, chunk):
    """Round-robin: per-gather Q7 pair cost is queue-independent (HW
    measured), and every instruction needs all 8 cores to at least
    idle-respond, so unequal shares only serialize the stream."""
    mode = os.environ.get("KERNEL_QSCHED", "rr")
    if mode == "rr":
        return [i % N_SWDGE_QUEUES for i in range(n_gathers)]
    costs = [queue_cost_ns(q, chunk) for q in range(N_SWDGE_QUEUES)]
    load = [0.0] * N_SWDGE_QUEUES
    sched = []
    for _ in range(n_gathers):
        q = min(range(N_SWDGE_QUEUES), key=lambda k: load[k] + costs[k])
        load[q] += costs[q]
        sched.append(q)
    return sched


def build_nc(group_counts=None):
    """group_counts: optional per-group valid-index counts (len N_GROUPS,
    SPMD-uniform = max over cores). Slots beyond the count carry index -1;
    the dma_gather ucode trims trailing negatives, skipping their
    descriptors and DMA reads. The count must be passed via num_idxs_reg so
    the sequencer's ring bookkeeping matches the ucode's descriptor count.
    """
    nc = bass.Bass(
        num_devices=N_CORES,
        num_swdge_queues=N_SWDGE_QUEUES,
        dynamic_dma_scratch_size=DMA_SCRATCH,
    )
    h_dt = mybir.dt.bfloat16 if H_BF16 else mybir.dt.float32
    h = nc.dram_tensor("h", [N_NODES, D], h_dt, kind="ExternalInput")
    idx = nc.dram_tensor(
        "idx", [P, N_GATHERS * IDX_COLS], mybir.dt.int16, kind="ExternalInput"
    )
    sc_out = nc.dram_tensor("sc", [P, SCORE_COLS], mybir.dt.float32,
                            kind="ExternalOutput")

    with tile.TileContext(nc) as tc:
        with (
            tc.tile_pool(name="io", bufs=1) as io_pool,
            tc.tile_pool(name="gs", bufs=GBUFS) as gs_pool,
            tc.tile_pool(name="gd", bufs=GBUFS) as gd_pool,
            tc.tile_pool(name="m", bufs=MBUFS) as m_pool,
        ):
            nc.gpsimd.load_library(library_config.mlp)
            if group_counts is None:
                chunk_counts = [CHUNK] * N_CHUNKS
            else:
                # chunk ci covers slots [ci*CHUNK, (ci+1)*CHUNK) of group
                # ci*CHUNK//GROUP_CAP; valid idxs remaining in this chunk:
                chunk_counts = [
                    max(0, min(CHUNK,
                               group_counts[ci * CHUNK // GROUP_CAP]
                               - (ci * CHUNK) % GROUP_CAP))
                    for ci in range(N_CHUNKS)
                ]
            nidx_regs = {n: nc.gpsimd.to_reg(int(n))
                         for n in sorted(set(chunk_counts))}
            # Optionally split the idx upload so early gathers don't wait
            # for the full 2.6MB transfer (measured neutral-to-negative on
            # HW, so default 1).
            idx_tiles = []  # (tile, col_base, col_end)
            n_splits = IDX_SPLITS
            cols_per = (N_GATHERS // n_splits) * IDX_COLS
            for si in range(n_splits):
                c0 = si * cols_per
                c1 = (si + 1) * cols_per if si < n_splits - 1 else (
                    N_GATHERS * IDX_COLS)
                t = io_pool.tile([P, c1 - c0], mybir.dt.int16)
                nc.sync.dma_start(out=t[:], in_=idx[:, c0:c1])
                idx_tiles.append((t, c0, c1))

            def idx_slice(gi):
                c0, c1 = gi * IDX_COLS, (gi + 1) * IDX_COLS
                for t, b0, b1 in idx_tiles:
                    if b0 <= c0 and c1 <= b1:
                        return t[:, c0 - b0 : c1 - b0]
                raise AssertionError((gi, c0, c1))

            scores = io_pool.tile([P, SCORE_COLS], mybir.dt.float32)
            if SKIP_DVE:
                nc.vector.memset(scores[:], 0.0)

            qsched = build_queue_schedule(N_GATHERS, CHUNK)
            for ci in range(N_CHUNKS):
                grp = ci * CHUNK // GROUP_CAP
                bs, bd = grp // N_BANKS, grp % N_BANKS
                gs = gs_pool.tile([P, CHUNK], h_dt, tag="gs")
                gd = gd_pool.tile([P, CHUNK], h_dt, tag="gd")
                for side, (g_tile, bank) in enumerate([(gs, bs), (gd, bd)]):
                    gi = 2 * ci + side
                    nc.gpsimd.dma_gather(
                        out_ap=g_tile[:].rearrange("p (b e) -> p b e", e=D),
                        in_ap=h[bank * BANK : (bank + 1) * BANK, :],
                        idxs_ap=idx_slice(gi),
                        num_idxs=CHUNK,
                        num_idxs_reg=nidx_regs[chunk_counts[ci]],
                        elem_size=D,
                        single_packet=SINGLE_PACKET,
                        queue_num=qsched[gi],
                    )
                if SKIP_DVE:
                    continue
                # Slice the DVE work: each 2-input tensor_tensor holds the
                # shared SBUF port pair (blocking Q7 descriptor generation)
                # for its whole duration, so keep the lock windows short.
                m_dt = mybir.dt.bfloat16 if M_BF16 else mybir.dt.float32
                for si in range(CHUNK // DVE_W):
                    sl = slice(si * DVE_W, (si + 1) * DVE_W)
                    m = m_pool.tile([P, DVE_W], m_dt, tag="m")
                    nc.vector.tensor_tensor(
                        out=m[:], in0=gs[:, sl], in1=gd[:, sl],
                        op=mybir.AluOpType.mult,
                    )
                    nc.vector.tensor_reduce(
                        out=scores[
                            :,
                            ci * CB + si * (DVE_W // P)
                            : ci * CB + (si + 1) * (DVE_W // P),
                        ],
                        in_=m[:].rearrange("p (b e) -> p b e", e=D),
                        axis=mybir.AxisListType.X,
                        op=mybir.AluOpType.add,
                    )

            # Raw scores only; the global min + (score != min) threshold are
            # O(E) elementwise host work in assemble_output.
            nc.sync.dma_start(out=sc_out[:], in_=scores[:])

    _split_multi_waits(nc)
    # populate .instr bytes of InstISA subclasses (the library-reload pseudo);
    # raw Bass skips this Bacc pass and walrus errors "ISA wrong length"
    mybir.codegen_inst_isa_subclasses(nc)
    return nc


def _split_multi_waits(nc):
    """walrus on this compiler rejects >1 sync-wait command per ISA
    instruction (setupSyncWait: "Too many sync wait commands"). Move all but
    one wait off each instruction onto standalone InstEventSemaphore
    instructions placed immediately before it on the same engine — the
    sequencer blocks on those first, which is semantically identical."""
    n = 0
    for b in nc.m.functions[0].blocks:
        new_list = []
        for ins in b.instructions:
            si = ins.sync_info
            if (
                si is not None
                and si.on_wait
                and len(si.on_wait) > 1
                and not isinstance(ins, mybir.InstEventSemaphore)
            ):
                waits = list(si.on_wait)
                for w in waits[:-1]:
                    n += 1
                    ev = mybir.InstEventSemaphore(
                        name=f"wait_split_{n}",
                        opcode="EventSemaphore",
                        engine=ins.engine,
                        ins=[],
                        outs=[],
                        sync_info=mybir.SyncInfo(on_wait=[w], on_update=[]),
                    )
                    nc.inst_map[ev.name] = ev
                    new_list.append(ev)
                si.on_wait = [waits[-1]]
            new_list.append(ins)
        b.instructions[:] = new_list


def group_counts_of(src, dst):
    gkey = (src // BANK) * N_BANKS + (dst // BANK)
    return np.bincount(gkey, minlength=N_GROUPS)


def _plan_core(src, dst, n_pad):
    """Group this core's edges by (src_bank, dst_bank) with fixed caps.

    n_pad[g]: SPMD-uniform valid-slot count per group (>= this core's
    count, <= GROUP_CAP). Slots [count, n_pad[g]) are duplicate edges
    (gathered but never extracted); slots [n_pad[g], GROUP_CAP) get index
    -1, which the dma_gather ucode trims (no descriptor, no DMA read).

    Returns (idx16 [P, N_GATHERS*IDX_COLS], slot_of_edge [n], overflow)."""
    n = src.shape[0]
    gkey = (src // BANK) * N_BANKS + (dst // BANK)
    order = np.argsort(gkey, kind="stable")
    counts = np.bincount(gkey, minlength=N_GROUPS)
    overflow = []
    starts = np.zeros(N_GROUPS + 1, np.int64)
    np.cumsum(counts, out=starts[1:])
    src_slots = np.empty(SLOTS, np.int32)  # bank-local src index per slot
    dst_slots = np.empty(SLOTS, np.int32)
    slot_of_edge = np.full(n, -1, np.int64)
    for g in range(N_GROUPS):
        bs, bd = g // N_BANKS, g % N_BANKS
        members = order[starts[g] : starts[g + 1]]
        if len(members) > GROUP_CAP:
            for pos in members[GROUP_CAP:]:
                overflow.append(int(pos))
            members = members[:GROUP_CAP]
        base = g * GROUP_CAP
        k = len(members)
        npad = max(int(n_pad[g]), k)
        slot_of_edge[members] = base + np.arange(k)
        sv = src[members] - bs * BANK
        dv = dst[members] - bd * BANK
        pad_s, pad_d = (sv[0], dv[0]) if k else (0, 0)
        src_slots[base : base + k] = sv
        src_slots[base + k : base + npad] = pad_s
        src_slots[base + npad : base + GROUP_CAP] = -1
        dst_slots[base : base + k] = dv
        dst_slots[base + k : base + npad] = pad_d
        dst_slots[base + npad : base + GROUP_CAP] = -1
    # build idx16: gather gi=2*ci covers src of chunk ci, gi=2*ci+1 dst
    idx16 = np.empty((16, N_GATHERS * IDX_COLS), np.int16)
    for ci in range(N_CHUNKS):
        for side, arr in ((0, src_slots), (1, dst_slots)):
            gi = 2 * ci + side
            vals = arr[ci * CHUNK : (ci + 1) * CHUNK]
            # index i lives at [i % 16, i // 16]
            idx16[:, gi * IDX_COLS : (gi + 1) * IDX_COLS] = (
                vals.reshape(IDX_COLS, 16).T
            )
    idx16_full = np.tile(idx16, (8, 1))  # replicate across the 8 Q7 cores
    return idx16_full, slot_of_edge, overflow


def refresh_layout():
    """(Re)build padded-slot -> (row, col) maps for the [P, SCORE_COLS]
    outputs. Called at import; call again if module constants are overridden
    (scaled-down tests)."""
    global _ROW_OF_SLOT, _COL_OF_SLOT
    s = np.arange(SLOTS)
    _ROW_OF_SLOT = (s % CHUNK % P).astype(np.int64)
    _COL_OF_SLOT = ((s // CHUNK) * CB + (s % CHUNK) // P).astype(np.int64)


refresh_layout()


def make_in_maps(h, src, dst):
    if H_BF16:
        import ml_dtypes
        h32 = np.ascontiguousarray(
            np.asarray(h, dtype=np.float32).astype(ml_dtypes.bfloat16)
        )
    else:
        h32 = np.ascontiguousarray(np.asarray(h, dtype=np.float32))
    src32 = np.asarray(src, dtype=np.int64)
    dst32 = np.asarray(dst, dtype=np.int64)
    per_core = [
        (src32[c * EPC : (c + 1) * EPC], dst32[c * EPC : (c + 1) * EPC])
        for c in range(N_CORES)
    ]
    if os.environ.get("KERNEL_TRIM", "1") == "1":
        # SPMD-uniform per-group valid counts: max over cores, capped
        n_pad = np.max(
            [group_counts_of(s, d) for s, d in per_core], axis=0
        ).clip(max=GROUP_CAP)
    else:
        n_pad = np.full(N_GROUPS, GROUP_CAP, np.int64)
    in_maps, plans = [], []
    for s, d in per_core:
        idx16, slot_of_edge, overflow = _plan_core(s, d, n_pad)
        in_maps.append({"h": h32, "idx": np.ascontiguousarray(idx16)})
        plans.append((slot_of_edge, overflow, s, d, False))
    return in_maps, plans, n_pad


def assemble_output(results, plans, h):
    # Device ships raw f32 scores; min + threshold are trivial elementwise
    # host work (same class as the slot-layout unpermutation below).
    h32 = None
    gmin = np.inf
    core_scores = []
    for (slot_of_edge, overflow, s, d, _), r in zip(plans, results):
        sc = r["sc"][_ROW_OF_SLOT[slot_of_edge], _COL_OF_SLOT[slot_of_edge]]
        if overflow:
            if h32 is None:
                h32 = np.asarray(h, dtype=np.float32)
            for pos in overflow:
                sc[pos] = float(h32[s[pos]] @ h32[d[pos]])
        core_scores.append(sc)
        gmin = min(gmin, float(sc.min()))
    outs = [(sc != gmin).astype(np.float32) for sc in core_scores]
    return np.concatenate(outs).reshape(E_TOTAL, 1).astype(np.float32)


def kernel(h, src, dst):
    in_maps, plans, n_pad = make_in_maps(h, src, dst)
    key = tuple(int(x) for x in n_pad)
    if _CACHE.get("key") != key:
        _CACHE["nc"] = build_nc(group_counts=key)
        _CACHE["key"] = key
    res = run_bass_kernel_spmd(_CACHE["nc"], in_maps, list(range(N_CORES)))
    return assemble_output(res.results, plans, h)



# revision 31
# speedup vs baseline: 1.0042x; 1.0042x over previous
"""DotProductPredictor kernel for trn2 (8 NeuronCores, SPMD).

Computes per-edge dot products score[e] = <h[src[e]], h[dst[e]]> over 600k
edges against a 100k x 128 fp32 node table; the final output
(score != global_min(score)) as float32 [600000, 1] matches the
reference's min-max normalize + (norm==0 ? 0 : 1) threshold.

Device strategy: edges sharded 8-way data-parallel; h replicated. Row
gathers use the GPSIMD dma_gather custom instruction (int16 indices), so h
is split into 4 banks of 25000 rows and each core's edges are grouped by
(src_bank, dst_bank) into 16 groups of a fixed 5120-edge capacity. Pads up
to the SPMD-uniform per-group count are duplicate in-group edges; slots
beyond it carry index -1, which the ucode trims (no descriptor / no DMA
read). Gathers are spread round-robin over 4 SWDGE queues (per-gather Q7
pair cost is ~6ns/idx + ~2.6us fixed, queue-independent; every instruction
also needs all 8 Q7 cores to idle-respond, so equal shares win). h is cast
to bf16 host-side (min-gap 2.67 dwarfs bf16 noise ~0.05; the threshold
compares our own f32 scores to their own min, so the argmin is preserved).
DVE multiply + per-edge segmented reduce produce f32 scores; the device
ships raw scores and the host does the global min + (score != min)
threshold (plus the slot unpermutation it already did).
"""

import os

import numpy as np

from concourse import bass, mybir, tile
from concourse import library_config
from concourse.bass_utils import run_bass_kernel_spmd

P = 128            # SBUF partitions
D = 128            # feature dim (one h row = 512B)
N_NODES = 100000
E_TOTAL = 600000
N_CORES = 8
EPC = E_TOTAL // N_CORES       # 75000 edges per core

N_BANKS = 4
BANK = N_NODES // N_BANKS      # 25000 rows per bank (< 32768 => int16 ok)
N_GROUPS = N_BANKS * N_BANKS   # 16 (src_bank, dst_bank) groups
GROUP_CAP = 5120               # fixed per-group slot allocation (mean 4687)
# Per-gather Q7 pair cost is ~6ns/idx + ~2.6us fixed (HW-measured). Bigger
# chunks would amortize the fixed cost, but the dma_gather ucode stack
# breaks above 1024 indices/instruction (device hangs at 2048 and 5120,
# with either packeting mode and bigger descriptor rings).
CHUNK = int(os.environ.get("KERNEL_CHUNK", "1024"))
CB = CHUNK // P                # score blocks per chunk
N_CHUNKS = N_GROUPS * GROUP_CAP // CHUNK
SLOTS = N_GROUPS * GROUP_CAP   # 81920 padded edge slots per core
SCORE_COLS = SLOTS // P        # 640
IDX_COLS = CHUNK // 16         # int16 columns per gather
N_GATHERS = 2 * N_CHUNKS
DVE_W = int(os.environ.get("KERNEL_DVE_W", "1024"))  # DVE op width (lock window)

_CACHE = {}


N_SWDGE_QUEUES = int(os.environ.get("KERNEL_SWDGE_QUEUES", "4"))
H_BF16 = os.environ.get("KERNEL_H_BF16", "1") == "1"
# Per-descriptor packets: the concatenated-stream mode would put 65 descs
# in one packet (over the documented 64-desc ceiling) and measured slightly
# slower; per-desc packets also interleave better across the 4 queues.
SINGLE_PACKET = os.environ.get("KERNEL_SINGLE_PACKET", "0") == "1"
DMA_SCRATCH = int(os.environ.get("KERNEL_DMA_SCRATCH", "16384"))
GBUFS = int(os.environ.get("KERNEL_GBUFS", "4"))
MBUFS = int(os.environ.get("KERNEL_MBUFS", "4"))
IDX_SPLITS = int(os.environ.get("KERNEL_IDX_SPLITS", "1"))
# bf16 product tile: 16-bit tensor_tensor runs at 2x on DVE, halving the
# window during which the DVE 2-input op holds the shared SBUF port pair
# that Q7 (SWDGE descriptor generation) needs. BUT a bf16-input
# tensor_reduce then runs in 16-bit 2x perf mode which ALSO grabs the
# shared pair (f32-input reduce does not), a net loss — so default off.
M_BF16 = os.environ.get("KERNEL_M_BF16", "0") == "1"
# Diagnostic: skip all DVE work to measure pure gather throughput.
SKIP_DVE = os.environ.get("KERNEL_SKIP_DVE", "0") == "1"

# Per-queue gather cost model (ns): the dma_gather ucode reads its int16
# index tile with channels=(queue+1)*32 active partitions starting at
# partition 0 (covering up to the issuing Q7 pair), so higher queue numbers
# pay a linearly larger idx-drain. Measured on HW (CHUNK=1024, no DVE):
# T(q) = B + KAPPA * 32*(q+1) * (CHUNK/16).
Q_BASE_NS = float(os.environ.get("KERNEL_QBASE", "2570"))
Q_KAPPA = float(os.environ.get("KERNEL_QKAPPA", "0.579"))


def queue_cost_ns(q, chunk):
    return Q_BASE_NS + Q_KAPPA * 32 * (q + 1) * (chunk / 16)


def build_queue_schedule(n_gathers, chunk):
    """Round-robin: per-gather Q7 pair cost is queue-independent (HW
    measured), and every instruction needs all 8 cores to at least
    idle-respond, so unequal shares only serialize the stream."""
    mode = os.environ.get("KERNEL_QSCHED", "rr")
    if mode == "rr":
        return [i % N_SWDGE_QUEUES for i in range(n_gathers)]
    costs = [queue_cost_ns(q, chunk) for q in range(N_SWDGE_QUEUES)]
    load = [0.0] * N_SWDGE_QUEUES
    sched = []
    for _ in range(n_gathers):
        q = min(range(N_SWDGE_QUEUES), key=lambda k: load[k] + costs[k])
        load[q] += costs[q]
        sched.append(q)
    return sched


def build_nc(group_counts=None):
    """group_counts: optional per-group valid-index counts (len N_GROUPS,
    SPMD-uniform = max over cores). Slots beyond the count carry index -1;
    the dma_gather ucode trims trailing negatives, skipping their
    descriptors and DMA reads. The count must be passed via num_idxs_reg so
    the sequencer's ring bookkeeping matches the ucode's descriptor count.
    """
    nc = bass.Bass(
        num_devices=N_CORES,
        num_swdge_queues=N_SWDGE_QUEUES,
        dynamic_dma_scratch_size=DMA_SCRATCH,
    )
    h_dt = mybir.dt.bfloat16 if H_BF16 else mybir.dt.float32
    h = nc.dram_tensor("h", [N_NODES, D], h_dt, kind="ExternalInput")
    idx = nc.dram_tensor(
        "idx", [P, N_GATHERS * IDX_COLS], mybir.dt.int16, kind="ExternalInput"
    )
    sc_out = nc.dram_tensor("sc", [P, SCORE_COLS], mybir.dt.float32,
                            kind="ExternalOutput")

    with tile.TileContext(nc) as tc:
        with (
            tc.tile_pool(name="io", bufs=1) as io_pool,
            tc.tile_pool(name="gs", bufs=GBUFS) as gs_pool,
            tc.tile_pool(name="gd", bufs=GBUFS) as gd_pool,
            tc.tile_pool(name="m", bufs=MBUFS) as m_pool,
        ):
            # Issue the idx upload (Sync/HWDGE) before the GPSIMD library
            # load so the ~6us IRAM load overlaps the ~9us idx transfer
            # instead of serializing ahead of it.
            idx_tiles = []  # (tile, col_base, col_end)
            n_splits = IDX_SPLITS
            cols_per = (N_GATHERS // n_splits) * IDX_COLS
            for si in range(n_splits):
                c0 = si * cols_per
                c1 = (si + 1) * cols_per if si < n_splits - 1 else (
                    N_GATHERS * IDX_COLS)
                t = io_pool.tile([P, c1 - c0], mybir.dt.int16)
                nc.sync.dma_start(out=t[:], in_=idx[:, c0:c1])
                idx_tiles.append((t, c0, c1))

            nc.gpsimd.load_library(library_config.mlp)
            if group_counts is None:
                chunk_counts = [CHUNK] * N_CHUNKS
            else:
                # chunk ci covers slots [ci*CHUNK, (ci+1)*CHUNK) of group
                # ci*CHUNK//GROUP_CAP; valid idxs remaining in this chunk:
                chunk_counts = [
                    max(0, min(CHUNK,
                               group_counts[ci * CHUNK // GROUP_CAP]
                               - (ci * CHUNK) % GROUP_CAP))
                    for ci in range(N_CHUNKS)
                ]
            nidx_regs = {n: nc.gpsimd.to_reg(int(n))
                         for n in sorted(set(chunk_counts))}
            def idx_slice(gi):
                c0, c1 = gi * IDX_COLS, (gi + 1) * IDX_COLS
                for t, b0, b1 in idx_tiles:
                    if b0 <= c0 and c1 <= b1:
                        return t[:, c0 - b0 : c1 - b0]
                raise AssertionError((gi, c0, c1))

            scores = io_pool.tile([P, SCORE_COLS], mybir.dt.float32)
            if SKIP_DVE:
                nc.vector.memset(scores[:], 0.0)

            qsched = build_queue_schedule(N_GATHERS, CHUNK)
            for ci in range(N_CHUNKS):
                grp = ci * CHUNK // GROUP_CAP
                bs, bd = grp // N_BANKS, grp % N_BANKS
                gs = gs_pool.tile([P, CHUNK], h_dt, tag="gs")
                gd = gd_pool.tile([P, CHUNK], h_dt, tag="gd")
                for side, (g_tile, bank) in enumerate([(gs, bs), (gd, bd)]):
                    gi = 2 * ci + side
                    nc.gpsimd.dma_gather(
                        out_ap=g_tile[:].rearrange("p (b e) -> p b e", e=D),
                        in_ap=h[bank * BANK : (bank + 1) * BANK, :],
                        idxs_ap=idx_slice(gi),
                        num_idxs=CHUNK,
                        num_idxs_reg=nidx_regs[chunk_counts[ci]],
                        elem_size=D,
                        single_packet=SINGLE_PACKET,
                        queue_num=qsched[gi],
                    )
                if SKIP_DVE:
                    continue
                # Slice the DVE work: each 2-input tensor_tensor holds the
                # shared SBUF port pair (blocking Q7 descriptor generation)
                # for its whole duration, so keep the lock windows short.
                m_dt = mybir.dt.bfloat16 if M_BF16 else mybir.dt.float32
                for si in range(CHUNK // DVE_W):
                    sl = slice(si * DVE_W, (si + 1) * DVE_W)
                    m = m_pool.tile([P, DVE_W], m_dt, tag="m")
                    nc.vector.tensor_tensor(
                        out=m[:], in0=gs[:, sl], in1=gd[:, sl],
                        op=mybir.AluOpType.mult,
                    )
                    nc.vector.tensor_reduce(
                        out=scores[
                            :,
                            ci * CB + si * (DVE_W // P)
                            : ci * CB + (si + 1) * (DVE_W // P),
                        ],
                        in_=m[:].rearrange("p (b e) -> p b e", e=D),
                        axis=mybir.AxisListType.X,
                        op=mybir.AluOpType.add,
                    )

            # Raw scores only; the global min + (score != min) threshold are
            # O(E) elementwise host work in assemble_output.
            nc.sync.dma_start(out=sc_out[:], in_=scores[:])

    _split_multi_waits(nc)
    # populate .instr bytes of InstISA subclasses (the library-reload pseudo);
    # raw Bass skips this Bacc pass and walrus errors "ISA wrong length"
    mybir.codegen_inst_isa_subclasses(nc)
    return nc


def _split_multi_waits(nc):
    """walrus on this compiler rejects >1 sync-wait command per ISA
    instruction (setupSyncWait: "Too many sync wait commands"). Move all but
    one wait off each instruction onto standalone InstEventSemaphore
    instructions placed immediately before it on the same engine — the
    sequencer blocks on those first, which is semantically identical."""
    n = 0
    for b in nc.m.functions[0].blocks:
        new_list = []
        for ins in b.instructions:
            si = ins.sync_info
            if (
                si is not None
                and si.on_wait
                and len(si.on_wait) > 1
                and not isinstance(ins, mybir.InstEventSemaphore)
            ):
                waits = list(si.on_wait)
                for w in waits[:-1]:
                    n += 1
                    ev = mybir.InstEventSemaphore(
                        name=f"wait_split_{n}",
                        opcode="EventSemaphore",
                        engine=ins.engine,
                        ins=[],
                        outs=[],
                        sync_info=mybir.SyncInfo(on_wait=[w], on_update=[]),
                    )
                    nc.inst_map[ev.name] = ev
                    new_list.append(ev)
                si.on_wait = [waits[-1]]
            new_list.append(ins)
        b.instructions[:] = new_list


def group_counts_of(src, dst):
    gkey = (src // BANK) * N_BANKS + (dst // BANK)
    return np.bincount(gkey, minlength=N_GROUPS)


def _plan_core(src, dst, n_pad):
    """Group this core's edges by (src_bank, dst_bank) with fixed caps.

    n_pad[g]: SPMD-uniform valid-slot count per group (>= this core's
    count, <= GROUP_CAP). Slots [count, n_pad[g]) are duplicate edges
    (gathered but never extracted); slots [n_pad[g], GROUP_CAP) get index
    -1, which the dma_gather ucode trims (no descriptor, no DMA read).

    Returns (idx16 [P, N_GATHERS*IDX_COLS], slot_of_edge [n], overflow)."""
    n = src.shape[0]
    gkey = (src // BANK) * N_BANKS + (dst // BANK)
    order = np.argsort(gkey, kind="stable")
    counts = np.bincount(gkey, minlength=N_GROUPS)
    overflow = []
    starts = np.zeros(N_GROUPS + 1, np.int64)
    np.cumsum(counts, out=starts[1:])
    src_slots = np.empty(SLOTS, np.int32)  # bank-local src index per slot
    dst_slots = np.empty(SLOTS, np.int32)
    slot_of_edge = np.full(n, -1, np.int64)
    for g in range(N_GROUPS):
        bs, bd = g // N_BANKS, g % N_BANKS
        members = order[starts[g] : starts[g + 1]]
        if len(members) > GROUP_CAP:
            for pos in members[GROUP_CAP:]:
                overflow.append(int(pos))
            members = members[:GROUP_CAP]
        base = g * GROUP_CAP
        k = len(members)
        npad = max(int(n_pad[g]), k)
        slot_of_edge[members] = base + np.arange(k)
        sv = src[members] - bs * BANK
        dv = dst[members] - bd * BANK
        pad_s, pad_d = (sv[0], dv[0]) if k else (0, 0)
        src_slots[base : base + k] = sv
        src_slots[base + k : base + npad] = pad_s
        src_slots[base + npad : base + GROUP_CAP] = -1
        dst_slots[base : base + k] = dv
        dst_slots[base + k : base + npad] = pad_d
        dst_slots[base + npad : base + GROUP_CAP] = -1
    # build idx16: gather gi=2*ci covers src of chunk ci, gi=2*ci+1 dst
    idx16 = np.empty((16, N_GATHERS * IDX_COLS), np.int16)
    for ci in range(N_CHUNKS):
        for side, arr in ((0, src_slots), (1, dst_slots)):
            gi = 2 * ci + side
            vals = arr[ci * CHUNK : (ci + 1) * CHUNK]
            # index i lives at [i % 16, i // 16]
            idx16[:, gi * IDX_COLS : (gi + 1) * IDX_COLS] = (
                vals.reshape(IDX_COLS, 16).T
            )
    idx16_full = np.tile(idx16, (8, 1))  # replicate across the 8 Q7 cores
    return idx16_full, slot_of_edge, overflow


def refresh_layout():
    """(Re)build padded-slot -> (row, col) maps for the [P, SCORE_COLS]
    outputs. Called at import; call again if module constants are overridden
    (scaled-down tests)."""
    global _ROW_OF_SLOT, _COL_OF_SLOT
    s = np.arange(SLOTS)
    _ROW_OF_SLOT = (s % CHUNK % P).astype(np.int64)
    _COL_OF_SLOT = ((s // CHUNK) * CB + (s % CHUNK) // P).astype(np.int64)


refresh_layout()


def make_in_maps(h, src, dst):
    if H_BF16:
        import ml_dtypes
        h32 = np.ascontiguousarray(
            np.asarray(h, dtype=np.float32).astype(ml_dtypes.bfloat16)
        )
    else:
        h32 = np.ascontiguousarray(np.asarray(h, dtype=np.float32))
    src32 = np.asarray(src, dtype=np.int64)
    dst32 = np.asarray(dst, dtype=np.int64)
    per_core = [
        (src32[c * EPC : (c + 1) * EPC], dst32[c * EPC : (c + 1) * EPC])
        for c in range(N_CORES)
    ]
    if os.environ.get("KERNEL_TRIM", "1") == "1":
        # SPMD-uniform per-group valid counts: max over cores, capped
        n_pad = np.max(
            [group_counts_of(s, d) for s, d in per_core], axis=0
        ).clip(max=GROUP_CAP)
    else:
        n_pad = np.full(N_GROUPS, GROUP_CAP, np.int64)
    in_maps, plans = [], []
    for s, d in per_core:
        idx16, slot_of_edge, overflow = _plan_core(s, d, n_pad)
        in_maps.append({"h": h32, "idx": np.ascontiguousarray(idx16)})
        plans.append((slot_of_edge, overflow, s, d, False))
    return in_maps, plans, n_pad


def assemble_output(results, plans, h):
    # Device ships raw f32 scores; min + threshold are trivial elementwise
    # host work (same class as the slot-layout unpermutation below).
    h32 = None
    gmin = np.inf
    core_scores = []
    for (slot_of_edge, overflow, s, d, _), r in zip(plans, results):
        sc = r["sc"][_ROW_OF_SLOT[slot_of_edge], _COL_OF_SLOT[slot_of_edge]]
        if overflow:
            if h32 is None:
                h32 = np.asarray(h, dtype=np.float32)
            for pos in overflow:
                sc[pos] = float(h32[s[pos]] @ h32[d[pos]])
        core_scores.append(sc)
        gmin = min(gmin, float(sc.min()))
    outs = [(sc != gmin).astype(np.float32) for sc in core_scores]
    return np.concatenate(outs).reshape(E_TOTAL, 1).astype(np.float32)


def kernel(h, src, dst):
    in_maps, plans, n_pad = make_in_maps(h, src, dst)
    key = tuple(int(x) for x in n_pad)
    if _CACHE.get("key") != key:
        _CACHE["nc"] = build_nc(group_counts=key)
        _CACHE["key"] = key
    res = run_bass_kernel_spmd(_CACHE["nc"], in_maps, list(range(N_CORES)))
    return assemble_output(res.results, plans, h)

